# revision 6
# baseline (speedup 1.0000x reference)
"""GraphWaveNet kernel for Trainium2 (Bass/Tile), 8 NeuronCores.

Algorithm notes:
- The reference output is h[:, 0, :, -1]: only the last time step (t=11)
  survives the final 1x1 conv. The GCN layers do not mix time, so only
  t=11 of the conv stack is needed -> conv1 at t in {10,11}, conv2 at
  t=11, GCN on B=4 slices instead of B*T=48.
- GCN: A_norm @ (h @ W) + b = (A_norm @ h) @ W + b (commute). With
  Hs = dsq * h (dsq = deg^-1/2 with self loops), the aggregation is
  agg[n] = dsq[n] * (sum_{e->n} Hs[src_e] + Hs[n]).
- Gather: Hs tables live in HBM as node-rows [Np, 256] bf16 (4 slices x
  64 feats in columns). indirect_dma_start gathers 128 edge rows per
  instruction (dst-sorted edge order).
- Scatter: host-built one-hot P [128 edges, 32-node window] bf16 per
  chunk; TensorE matmul P^T @ msgs accumulates 32-aligned windows into
  f32 PSUM via start/stop chains.
"""

import sys

sys.path.insert(0, "/opt/trn_rl_repo")

import numpy as np
import ml_dtypes

B, T, N, FIN, H, E = 4, 12, 10000, 2, 64, 80000
NB = 79            # node blocks of 128
NP = NB * 128      # padded node count (10112)
D = 4 * H          # 256 = 4 slices x 64 feats
W32 = 32           # scatter window (32-aligned for PSUM tile_position)
EMAX = 128         # max edges per scatter chunk

_cache = {}


def _host_prep(x, edge_index, w1, b1, w2, b2, gw1, gb1, gw2, gb2, wo, bo):
    src = np.asarray(edge_index[0], dtype=np.int64)
    dst = np.asarray(edge_index[1], dtype=np.int64)

    deg = np.bincount(dst, minlength=N).astype(np.float64) + 1.0
    dsq = (deg ** -0.5).astype(np.float32)

    order = np.argsort(dst, kind="stable")
    src_s = src[order]
    dst_s = dst[order]

    cnt = np.bincount(dst, minlength=NP).astype(np.int64)
    starts = np.zeros(NP + 1, dtype=np.int64)
    starts[1:] = np.cumsum(cnt)

    # chunks: fixed 32-node windows; a window's edges split into <=EMAX runs
    chunks = []            # (q, e0, ne, first, last) per chunk
    blk_chunks = [[] for _ in range(NB)]
    for blk in range(NB):
        for q in range(4):
            g0 = blk * 128 + q * 32
            e0, e1 = int(starts[g0]), int(starts[g0 + 32])
            ne = e1 - e0
            nparts = max(1, (ne + EMAX - 1) // EMAX)
            for j in range(nparts):
                a = e0 + j * EMAX
                b_ = min(e0 + (j + 1) * EMAX, e1)
                blk_chunks[blk].append(len(chunks))
                chunks.append((q, a, b_ - a, j == 0, j == nparts - 1))
    nch = len(chunks)

    idx_host = np.zeros((128, nch), dtype=np.int32)
    P_host = np.zeros((128, nch * W32), dtype=np.float32)
    ci = 0
    for blk in range(NB):
        for c in blk_chunks[blk]:
            q, e0, ne, _, _ = chunks[c]
            g0 = (blk * 128) + q * 32
            if ne:
                idx_host[:ne, c] = src_s[e0:e0 + ne]
                P_host[np.arange(ne), c * W32 + (dst_s[e0:e0 + ne] - g0)] = 1.0
    P_host = P_host.astype(ml_dtypes.bfloat16)

    # conv input: per slice 8 rows: (t,c) for t in {9,10,11}, then 2 zero rows
    xt = np.zeros((4, 8, NP), dtype=np.float32)
    for s in range(B):
        for ti, t in enumerate((9, 10, 11)):
            for c in range(FIN):
                xt[s, 2 * ti + c, :N] = x[s, t, :, c]

    W1m = np.zeros((6, 64), dtype=np.float32)
    for k in range(3):
        for c in range(FIN):
            W1m[2 * k + c, :] = w1[:, c, 0, k]
    # A: t10 output (x rows t9..t11 = k0..k2); B: t11 output (rows shifted by 2)
    W1ab = np.zeros((8, 128), dtype=np.float32)
    W1ab[0:6, 0:64] = W1m
    W1ab[2:8, 64:128] = W1m

    W2m = np.zeros((128, 64), dtype=np.float32)
    W2m[:64, :] = w2[:, :, 0, 0].T
    W2m[64:, :] = w2[:, :, 0, 1].T

    b1_st = np.concatenate([b1, b1]).reshape(128, 1).astype(np.float32)
    b2_col = b2.reshape(64, 1).astype(np.float32)
    gb1_st = np.concatenate([gb1, gb1]).reshape(128, 1).astype(np.float32)
    gb2_st = np.concatenate([gb2, gb2]).reshape(128, 1).astype(np.float32)
    gwd1 = np.concatenate([gw1, gw1], axis=0).astype(ml_dtypes.bfloat16)  # [128,64]
    gwd2 = np.concatenate([gw2, gw2], axis=0).astype(ml_dtypes.bfloat16)
    wov = wo[0, :, 0, 0]
    wod = np.concatenate([wov, wov]).reshape(128, 1).astype(ml_dtypes.bfloat16)

    dsq_pad = np.ones(NP, dtype=np.float32)
    dsq_pad[:N] = dsq
    dsq_nb = dsq_pad.reshape(NB, 128).T.copy()

    ins = {
        "xt": xt, "W1ab": W1ab, "W2m": W2m, "b1s": b1_st, "b2c": b2_col,
        "gb1s": gb1_st, "gb2s": gb2_st, "gwd1": gwd1, "gwd2": gwd2,
        "wod": wod, "dsqnb": dsq_nb, "eidx": idx_host, "P": P_host,
    }
    return ins, blk_chunks, chunks, float(bo[0])


def _build(blk_chunks, chunks, bo_f, reps):
    from concourse import bass, bacc, tile
    from concourse.masks import make_identity
    import mybir

    f32, bf16, i32 = mybir.dt.float32, mybir.dt.bfloat16, mybir.dt.int32
    nch = len(chunks)

    nc = bacc.Bacc("TRN2", target_bir_lowering=False, debug=False, num_devices=8)

    ext = {}
    for name, shape, dt in [
        ("xt", [4, 8, NP], f32), ("W1ab", [8, 128], f32), ("W2m", [128, 64], f32),
        ("b1s", [128, 1], f32), ("b2c", [64, 1], f32),
        ("gb1s", [128, 1], f32), ("gb2s", [128, 1], f32),
        ("gwd1", [128, 64], bf16), ("gwd2", [128, 64], bf16),
        ("wod", [128, 1], bf16), ("dsqnb", [128, NB], f32),
        ("eidx", [128, nch], i32), ("P", [128, nch * W32], bf16),
    ]:
        ext[name] = nc.dram_tensor(name, shape, dt, kind="ExternalInput").ap()
    y_ext = nc.dram_tensor("y", [128, 4 * NB], f32, kind="ExternalOutput").ap()
    table0 = nc.dram_tensor("table0", [NP, D], bf16).ap()
    table1 = nc.dram_tensor("table1", [NP, D], bf16).ap()

    with tile.TileContext(nc) as tc:
        with tc.tile_pool(name="const", bufs=1) as cp, \
             tc.tile_pool(name="hs", bufs=1) as hp:
            ct = {}
            for name in ("W1ab", "W2m", "b1s", "b2c", "gb1s", "gb2s",
                         "gwd1", "gwd2", "wod", "dsqnb", "eidx", "P"):
                t = cp.tile(list(ext[name].shape), ext[name].dtype, tag=name)
                nc.sync.dma_start(t[:], ext[name][:])
                ct[name] = t
            ident = cp.tile([128, 128], bf16, tag="ident")
            make_identity(nc, ident[:])
            y_nb = cp.tile([128, 4 * NB], f32, tag="ynb")

            hs0 = hp.tile([128, NB * D], bf16, tag="hs0")
            hs1 = hp.tile([128, NB * D], bf16, tag="hs1")

            # ---- conv stage: table0 = dsq * relu(conv2(relu(conv1 x))) at t=11
            with tc.tile_pool(name="cv", bufs=3) as vp, \
                 tc.tile_pool(name="cvp", bufs=2, space="PSUM") as pp:
                for blk in range(NB):
                    ns = slice(blk * 128, (blk + 1) * 128)
                    for s in range(4):
                        xblk = vp.tile([8, 128], f32, tag="xb")
                        nc.sync.dma_start(xblk[:], ext["xt"][s, :, ns])
                        ph1 = pp.tile([128, 128], f32, tag="ph1", space="PSUM")
                        nc.tensor.matmul(ph1[0:64, :], lhsT=ct["W1ab"][:, 0:64],
                                         rhs=xblk[:], start=True, stop=True)
                        nc.tensor.matmul(ph1[64:128, :], lhsT=ct["W1ab"][:, 64:128],
                                         rhs=xblk[:], start=True, stop=True)
                        h1sb = vp.tile([128, 128], f32, tag="h1")
                        nc.scalar.activation(h1sb[:], ph1[:],
                                             mybir.ActivationFunctionType.Relu,
                                             bias=ct["b1s"][:, 0:1])
                        ph2 = pp.tile([64, 128], f32, tag="ph2", space="PSUM")
                        nc.tensor.matmul(ph2[:], lhsT=ct["W2m"][:, :], rhs=h1sb[:],
                                         start=True, stop=True)
                        h2bf = vp.tile([64, 128], bf16, tag="h2")
                        nc.scalar.activation(h2bf[:], ph2[:],
                                             mybir.ActivationFunctionType.Relu,
                                             bias=ct["b2c"][:, 0:1])
                        ptp = pp.tile([128, 64], bf16, tag="ptp", space="PSUM")
                        nc.tensor.transpose(ptp[:], h2bf[:], ident[0:64, 0:64])
                        nc.vector.tensor_scalar_mul(
                            hs0[:, blk * D + 64 * s: blk * D + 64 * (s + 1)],
                            ptp[:], ct["dsqnb"][:, blk:blk + 1])
                    nc.sync.dma_start(table0[ns, :], hs0[:, blk * D:(blk + 1) * D])

            # ---- GCN layers
            for rep in range(reps):
                for L in range(2):
                    tbl_in = table0 if L == 0 else table1
                    hs_cur = hs0 if L == 0 else hs1
                    gwd = ct["gwd1"] if L == 0 else ct["gwd2"]
                    gbs = ct["gb1s"] if L == 0 else ct["gb2s"]
                    with tc.tile_pool(name=f"gw{rep}_{L}", bufs=8) as gp, \
                         tc.tile_pool(name=f"gv{rep}_{L}", bufs=3) as wv, \
                         tc.tile_pool(name=f"gp{rep}_{L}", bufs=1, space="PSUM") as qp, \
                         tc.tile_pool(name=f"gq{rep}_{L}", bufs=2, space="PSUM") as qp2:
                        for blk in range(NB):
                            ns = slice(blk * 128, (blk + 1) * 128)
                            pba = qp.tile([64, D], f32, tag="pba", space="PSUM")
                            pbb = qp.tile([64, D], f32, tag="pbb", space="PSUM")
                            for c in blk_chunks[blk]:
                                q, e0, ne, first, last = chunks[c]
                                g = gp.tile([128, D], bf16, tag="g")
                                nc.gpsimd.indirect_dma_start(
                                    out=g[:], out_offset=None, in_=tbl_in[:],
                                    in_offset=bass.IndirectOffsetOnAxis(
                                        ap=ct["eidx"][:, c:c + 1], axis=0))
                                pb_half = pba if q < 2 else pbb
                                qq = q % 2
                                nc.tensor.matmul(
                                    pb_half[32 * qq:32 * (qq + 1), :],
                                    lhsT=ct["P"][:, c * W32:(c + 1) * W32],
                                    rhs=g[:], start=first, stop=last)
                            ta = wv.tile([128, D], f32, tag="ta")
                            nc.vector.tensor_add(ta[0:64, :], pba[:],
                                                 hs_cur[0:64, blk * D:(blk + 1) * D])
                            nc.vector.tensor_add(ta[64:128, :], pbb[:],
                                                 hs_cur[64:128, blk * D:(blk + 1) * D])
                            tsc = wv.tile([128, D], bf16, tag="tsc")
                            nc.vector.tensor_scalar_mul(tsc[:], ta[:],
                                                        ct["dsqnb"][:, blk:blk + 1])
                            for pr in range(2):
                                tp = qp2.tile([128, 128], bf16, tag="tp", space="PSUM")
                                nc.tensor.transpose(
                                    tp[:], tsc[:, 128 * pr:128 * (pr + 1)], ident[:])
                                tps = wv.tile([128, 128], bf16, tag="tps")
                                nc.vector.tensor_copy(tps[:], tp[:])
                                wp = qp2.tile([128, 128], f32, tag="wp", space="PSUM")
                                nc.tensor.matmul(wp[0:64, :], lhsT=gwd[0:64, :],
                                                 rhs=tps[0:64, :], start=True, stop=True)
                                nc.tensor.matmul(wp[64:128, :], lhsT=gwd[64:128, :],
                                                 rhs=tps[64:128, :], start=True, stop=True)
                                if L == 0:
                                    hn = wv.tile([128, 128], bf16, tag="hn")
                                    nc.scalar.activation(
                                        hn[:], wp[:],
                                        mybir.ActivationFunctionType.Relu,
                                        bias=gbs[:, 0:1])
                                    tb = qp2.tile([128, 128], bf16, tag="tb",
                                                 space="PSUM")
                                    nc.tensor.transpose(tb[:], hn[:], ident[:])
                                    nc.vector.tensor_scalar_mul(
                                        hs1[:, blk * D + 128 * pr:
                                            blk * D + 128 * (pr + 1)],
                                        tb[:], ct["dsqnb"][:, blk:blk + 1])
                                else:
                                    h4 = wv.tile([128, 128], bf16, tag="h4")
                                    nc.scalar.activation(
                                        h4[:], wp[:],
                                        mybir.ActivationFunctionType.Relu,
                                        bias=gbs[:, 0:1])
                                    for sl in range(2):
                                        yp = qp2.tile([128, 1], f32, tag="yp",
                                                     space="PSUM")
                                        nc.tensor.matmul(
                                            yp[:],
                                            lhsT=h4[64 * sl:64 * (sl + 1), :],
                                            rhs=ct["wod"][64 * sl:64 * (sl + 1), :],
                                            start=True, stop=True)
                                        nc.vector.tensor_scalar_add(
                                            y_nb[:, 4 * blk + 2 * pr + sl:
                                                 4 * blk + 2 * pr + sl + 1],
                                            yp[:], bo_f)
                            if L == 0:
                                nc.sync.dma_start(table1[ns, :],
                                                  hs1[:, blk * D:(blk + 1) * D])
            nc.sync.dma_start(y_ext[:], y_nb[:])
    nc.compile()
    return nc


def _run(inputs, reps=1):
    from concourse.bass_utils import run_bass_kernel_spmd

    ins, blk_chunks, chunks, bo_f = _host_prep(
        inputs["x"], inputs["edge_index"], inputs["w1"], inputs["b1"],
        inputs["w2"], inputs["b2"], inputs["gw1"], inputs["gb1"],
        inputs["gw2"], inputs["gb2"], inputs["wo"], inputs["bo"])

    key = (len(chunks), reps)
    if key not in _cache:
        _cache[key] = _build(blk_chunks, chunks, bo_f, reps)
    nc = _cache[key]

    in_maps = [dict(ins) for _ in range(8)]
    res = run_bass_kernel_spmd(nc, in_maps, list(range(8)))
    y_nb = res.results[0]["y"]          # [128, 4*NB]
    y = np.zeros((B, N), dtype=np.float32)
    for blk in range(NB):
        lo, hi = blk * 128, min((blk + 1) * 128, N)
        for s in range(B):
            y[s, lo:hi] = y_nb[: hi - lo, 4 * blk + s]
    return y


def kernel(**inputs):
    return _run(inputs, reps=1)
